# revision 1
# baseline (speedup 1.0000x reference)
"""MoE-routed transformer encoder layer on 8 Trainium2 cores.

Routing (mean -> nearest center -> expert id) is computed on host; sentences
are dispatched to cores so that each core runs exactly one expert's weights
over its share of sentences (expert/data parallelism, no device collectives).
The device kernel is a dense encoder layer: QKV -> attention -> out-proj ->
LN1 -> FFN(gelu) -> LN2, computed in fp32 with fp32r (full-rate) matmuls.
"""

import numpy as np

H = 768
NH = 12
HD = 64
FF = 3072
S = 128
E = 4
EPS = 1e-12
NCORES = 8

PARAM_KEYS = [
    "wq", "wk", "wv", "wo", "bq", "bk", "bv", "bo",
    "ln1_g", "ln1_b", "w1", "b1", "w2", "b2", "ln2_g", "ln2_b",
]

_BUILD_CACHE = {}
LAST_RUN_WALL_NS = None
_SIM_GELU_IDENTITY = False  # test-only: CoreSim has no gelu table
_STAGE = 2  # debug: 0=x->out copy, 1=phase A only, 2=full
_LOOP_R = 0  # debug: >0 wraps kernel body in a hardware loop (timing)
_SUB = 99  # debug sub-stage within phase A
_XT_F32 = False  # debug: xT in plain f32
_ATT_LVL = 4  # debug: 0=copy scores,1=+exp,2=+normalize,3=+transpose(full)


def _split_multi_waits(nc, mybir):
    # walrus in this env caps sync waits at 1 per CTRL-encoded instruction
    # (Drain); hoist extras onto single-wait InstDrain carriers inserted just
    # before the original. Compute/DMA instructions keep native multi-wait.
    for f in nc.m.functions:
        for b in f.blocks:
            insts = list(b.instructions)
            new, changed = [], False
            for inst in insts:
                si = inst.sync_info
                if (
                    isinstance(inst, mybir.InstDrain)
                    and si is not None
                    and len(si.on_wait) > 1
                ):
                    waits = list(si.on_wait)
                    for w in waits[:-1]:
                        d = mybir.InstDrain(
                            name=nc.get_next_instruction_name(), ins=[], outs=[]
                        )
                        d.engine = inst.engine
                        d.sync_info = mybir.SyncInfo(on_wait=[w], on_update=[])
                        nc.register_instruction(d)
                        new.append(d)
                    si.on_wait = [waits[-1]]
                    changed = True
                new.append(inst)
            if changed:
                b.instructions = new


def _build(nslot, use_mask):
    import concourse.bass as bass
    import concourse.mybir as mybir
    import concourse.tile as tile
    from concourse import bacc
    from concourse.masks import make_identity

    f32 = mybir.dt.float32
    f32r = mybir.dt.float32r
    AF = mybir.ActivationFunctionType
    ALU = mybir.AluOpType

    NS = nslot
    assert NS % 4 == 0
    G = NS // 4

    nc = bacc.Bacc("TRN2", target_bir_lowering=False, debug=False)

    x_d = nc.dram_tensor("x", [NS, S, H], f32, kind="ExternalInput").ap()
    mask_d = nc.dram_tensor("mask", [NS, S], f32, kind="ExternalInput").ap()
    wq_d = nc.dram_tensor("wq", [H, H], f32, kind="ExternalInput").ap()
    wk_d = nc.dram_tensor("wk", [H, H], f32, kind="ExternalInput").ap()
    wv_d = nc.dram_tensor("wv", [H, H], f32, kind="ExternalInput").ap()
    wo_d = nc.dram_tensor("wo", [H, H], f32, kind="ExternalInput").ap()
    bq_d = nc.dram_tensor("bq", [H], f32, kind="ExternalInput").ap()
    bk_d = nc.dram_tensor("bk", [H], f32, kind="ExternalInput").ap()
    bv_d = nc.dram_tensor("bv", [H], f32, kind="ExternalInput").ap()
    bo_d = nc.dram_tensor("bo", [H], f32, kind="ExternalInput").ap()
    g1_d = nc.dram_tensor("ln1_g", [H], f32, kind="ExternalInput").ap()
    b1l_d = nc.dram_tensor("ln1_b", [H], f32, kind="ExternalInput").ap()
    w1_d = nc.dram_tensor("w1", [H, FF], f32, kind="ExternalInput").ap()
    b1_d = nc.dram_tensor("b1", [FF], f32, kind="ExternalInput").ap()
    w2_d = nc.dram_tensor("w2", [FF, H], f32, kind="ExternalInput").ap()
    b2_d = nc.dram_tensor("b2", [H], f32, kind="ExternalInput").ap()
    g2_d = nc.dram_tensor("ln2_g", [H], f32, kind="ExternalInput").ap()
    b2l_d = nc.dram_tensor("ln2_b", [H], f32, kind="ExternalInput").ap()
    out_d = nc.dram_tensor("out", [NS, S, H], f32, kind="ExternalOutput").ap()

    x_sv = x_d.rearrange("n s h -> s n h")       # partition dim = sequence pos
    out_sv = out_d.rearrange("n s h -> s n h")

    def r(v):
        return v.bitcast(f32r)

    from contextlib import nullcontext

    with tile.TileContext(nc) as tc:
        with (tc.For_i(0, _LOOP_R, 1) if _LOOP_R > 0 else nullcontext()):
            _kernel_body(
                nc, tc, bass, mybir, tile, make_identity, NS, G, use_mask,
                x_sv, out_sv, mask_d,
                wq_d, wk_d, wv_d, wo_d, bq_d, bk_d, bv_d, bo_d,
                g1_d, b1l_d, w1_d, b1_d, w2_d, b2_d, g2_d, b2l_d,
            )
    nc.compile()
    return nc


def _kernel_body(nc, tc, bass, mybir, tile, make_identity, NS, G, use_mask,
                 x_sv, out_sv, mask_d,
                 wq_d, wk_d, wv_d, wo_d, bq_d, bk_d, bv_d, bo_d,
                 g1_d, b1l_d, w1_d, b1_d, w2_d, b2_d, g2_d, b2l_d):
    f32 = mybir.dt.float32
    f32r = mybir.dt.float32r
    AF = mybir.ActivationFunctionType
    ALU = mybir.AluOpType
    H = 768
    S = 128
    NH = 12
    EPS = 1e-12
    if True:
        with (
            tc.tile_pool(name="const", bufs=1) as constp,
            tc.tile_pool(name="ybuf", bufs=1) as ybufp,
        ):
            ident = constp.tile([128, 128], f32)
            make_identity(nc, ident)
            eps_t = constp.tile([128, 1], f32)
            nc.vector.memset(eps_t, EPS)
            b1_sb = constp.tile([128, 24], f32)
            nc.gpsimd.dma_start(b1_sb, b1_d.rearrange("(o p) -> p o", p=128))

            def repl(pool, src, nm):
                t = pool.tile([128, H], f32, tag=nm, name=nm)
                bsrc = bass.AP(
                    tensor=src.tensor, offset=src.offset, ap=[[0, 128], [1, H]]
                )
                nc.gpsimd.dma_start(t, bsrc)
                return t


            b2_r = repl(constp, b2_d, "b2_r")
            g2_r = repl(constp, g2_d, "g2_r")
            b2l_r = repl(constp, b2l_d, "b2l_r")
            y_all = ybufp.tile([128, NS, H], f32)
            yT_all = ybufp.tile([128, 6, NS, 128], mybir.dt.float32r)
            w1_view = w1_d.rearrange("(ko p) f -> p ko f", p=128)

            if _STAGE == 0:
                xt0 = ybufp.tile([128, NS, H], f32, tag="xt0", name="xt0")
                nc.sync.dma_start(xt0, x_sv)
                nc.sync.dma_start(out_sv, xt0)

            # ---------------- Phase A: attention + LN1 -> y_all ----------
            with (
                tc.tile_pool(name="pa", bufs=1) as pa,
                tc.tile_pool(name="pa2", bufs=2) as pa2,
                tc.tile_pool(name="pw", bufs=2) as pw,
                tc.tile_pool(name="psA_small", bufs=2, space="PSUM") as psAs,
                tc.tile_pool(name="psA_big", bufs=4, space="PSUM") as psAb,
                tc.tile_pool(name="psA_v", bufs=1, space="PSUM") as psAv,
            ):
                bq_sb = pa.tile([128, 6], f32, tag="bq_sb", name="bq_sb")
                nc.gpsimd.dma_start(bq_sb, bq_d.rearrange("(o p) -> p o", p=128))
                bk_sb = pa.tile([128, 6], f32, tag="bk_sb", name="bk_sb")
                nc.gpsimd.dma_start(bk_sb, bk_d.rearrange("(o p) -> p o", p=128))
                bv_r = repl(pa, bv_d, "bv_r")
                bo_r = repl(pa, bo_d, "bo_r")
                g1_r = repl(pa, g1_d, "g1_r")
                b1l_r = repl(pa, b1l_d, "b1l_r")
                for g in range(G if _STAGE >= 1 else 0):
                    s0 = g * 4
                    x_g = pa.tile([128, 4, H], f32, tag="x_g")
                    nc.sync.dma_start(x_g, x_sv[:, s0 : s0 + 4, :])
                    if use_mask:
                        mrep = pa.tile([128, 4, S], f32, tag="mrep")
                        src = bass.AP(
                            tensor=mask_d.tensor,
                            offset=s0 * S,
                            ap=[[0, 128], [S, 4], [1, S]],
                        )
                        nc.gpsimd.dma_start(mrep, src)

                    # x transposed: xT[p, c, si, s] = x[s, si, c*128+p]
                    xT = pa.tile([128, 6, 4, 128], f32 if _XT_F32 else f32r, tag="xT")
                    for si in range(4):
                        for c in range(6):
                            pt = psAs.tile([128, 128], f32, tag="pt")
                            nc.tensor.transpose(
                                pt, x_g[:, si, c * 128 : (c + 1) * 128], ident
                            )
                            nc.vector.tensor_copy(xT[:, c, si, :], pt)

                    if _SUB == 0:
                        ocp = pa.tile([128, 4, H], f32, tag="ocp", name="ocp")
                        nc.vector.tensor_copy(
                            ocp.rearrange("p n h -> p (n h)"),
                            xT.rearrange("p c n s -> p (c n s)").bitcast(f32),
                        )
                        nc.sync.dma_start(out_sv[:, s0 : s0 + 4, :], ocp)
                        continue

                    # qT/kT: weight-stationary over 4-sentence pack (N=512)
                    qT = pa.tile([128, 6, 4, 128], f32, tag="qT")
                    kT = pa.tile([128, 6, 4, 128], f32, tag="kT")
                    for w_dram, bias_sb, dstT in (
                        (wq_d, bq_sb, qT),
                        (wk_d, bk_sb, kT),
                    ):
                        w_sb = pw.tile([128, 6, H], f32r, tag="wqkvo")
                        nc.sync.dma_start(
                            w_sb,
                            w_dram.rearrange("(ko p) m -> p ko m", p=128).bitcast(f32r),
                        )
                        for mc in range(6):
                            pq = psAb.tile([128, 512], f32, tag="pq")
                            for kc in range(6):
                                nc.tensor.matmul(
                                    pq,
                                    w_sb[:, kc, mc * 128 : (mc + 1) * 128],
                                    xT[:, kc, :, :],
                                    start=(kc == 0),
                                    stop=(kc == 5),
                                )
                            nc.scalar.activation(
                                dstT[:, mc, :, :],
                                pq,
                                AF.Identity,
                                bias=bias_sb[:, mc : mc + 1],
                                scale=1.0,
                            )

                    if _SUB == 1:
                        nc.sync.dma_start(
                            out_sv[:, s0 : s0 + 4, :],
                            qT.rearrange("p c n s -> p (c n s)")
                            .rearrange("p (n h) -> p n h", n=4),
                        )
                        continue

                    # v in natural layout [s, 768]
                    wv_sb = pw.tile([128, 6, H], f32r, tag="wqkvo")
                    nc.sync.dma_start(
                        wv_sb,
                        wv_d.rearrange("(ko p) m -> p ko m", p=128).bitcast(f32r),
                    )
                    v_g = pa.tile([128, 4, H], f32, tag="v_g")
                    for si in range(4):
                        pv = psAv.tile([128, H], f32, tag="pv")
                        for kc in range(6):
                            nc.tensor.matmul(
                                pv[:, 0:512],
                                xT[:, kc, si, :],
                                wv_sb[:, kc, 0:512],
                                start=(kc == 0),
                                stop=(kc == 5),
                            )
                        for kc in range(6):
                            nc.tensor.matmul(
                                pv[:, 512:H],
                                xT[:, kc, si, :],
                                wv_sb[:, kc, 512:H],
                                start=(kc == 0),
                                stop=(kc == 5),
                            )
                        nc.vector.tensor_add(v_g[:, si, 0:512], pv[:, 0:512], bv_r[:, 0:512])
                        nc.vector.tensor_add(v_g[:, si, 512:H], pv[:, 512:H], bv_r[:, 512:H])

                    if _SUB == 2:
                        nc.sync.dma_start(out_sv[:, s0 : s0 + 4, :], v_g)
                        continue

                    # attention per sentence
                    ctxT = pa.tile([128, 6, 4, 128], f32r, tag="xT")  # reuse xT slot
                    for si in range(4):
                        attn = pa2.tile([128, NH, S], f32, tag="attn")
                        sums = pa2.tile([128, NH], f32, tag="sums")
                        for h in range(NH):
                            # one PSUM bank per head: a shared bank would be
                            # PE-written (next head) while read (this head),
                            # which is fatal on HW. Head pairs pack into the
                            # PE array (rows 0:64 / 64:128) and run
                            # concurrently via tile_position.
                            psc = psAb.tile([128, 128], f32, tag="pq", name="psc")
                            nc.tensor.matmul(
                                psc,
                                qT[(h % 2) * 64 : (h % 2) * 64 + 64, h // 2, si, :],
                                kT[(h % 2) * 64 : (h % 2) * 64 + 64, h // 2, si, :],
                                start=True,
                                stop=True,
                                tile_position=((h % 2) * 64, 0),
                            )
                            if _ATT_LVL == 0:
                                nc.vector.tensor_copy(attn[:, h, :], psc)
                            elif use_mask:
                                tmp = pa.tile([128, S], f32, tag="msk_tmp")
                                nc.vector.tensor_scalar_mul(tmp, psc, 0.125)
                                nc.vector.tensor_add(tmp, tmp, mrep[:, si, :])
                                nc.scalar.activation(
                                    attn[:, h, :], tmp, AF.Exp,
                                    bias=0.0, scale=1.0,
                                    accum_out=sums[:, h : h + 1],
                                )
                            else:
                                nc.scalar.activation(
                                    attn[:, h, :], psc, AF.Exp,
                                    bias=0.0, scale=0.125,
                                    accum_out=sums[:, h : h + 1],
                                )
                        if _ATT_LVL >= 2:
                            rs = pa2.tile([128, NH], f32, tag="rs")
                            nc.vector.reciprocal(rs, sums)
                            for h in range(NH):
                                nc.vector.tensor_scalar_mul(
                                    attn[:, h, :], attn[:, h, :], rs[:, h : h + 1]
                                )
                        attnT = pa2.tile([128, NH, S], f32, tag="attnT")
                        if _ATT_LVL >= 3:
                            for h in range(NH):
                                pt = psAs.tile([128, 128], f32, tag="pt")
                                nc.tensor.transpose(pt, attn[:, h, :], ident)
                                nc.vector.tensor_copy(attnT[:, h, :], pt)
                        else:
                            for h in range(NH):
                                nc.vector.tensor_copy(attnT[:, h, :], attn[:, h, :])
                        for hp in range(6):
                            pc = psAs.tile([128, 128], f32, tag="pt")
                            nc.tensor.matmul(
                                pc[0:64, :],
                                v_g[:, si, (2 * hp) * 64 : (2 * hp + 1) * 64],
                                attnT[:, 2 * hp, :],
                                start=True, stop=True,
                                tile_position=(0, 0),
                            )
                            nc.tensor.matmul(
                                pc[64:128, :],
                                v_g[:, si, (2 * hp + 1) * 64 : (2 * hp + 2) * 64],
                                attnT[:, 2 * hp + 1, :],
                                start=True, stop=True,
                                tile_position=(0, 64),
                            )
                            nc.vector.tensor_copy(ctxT[:, hp, si, :], pc)

                    if _SUB == 3:
                        nc.sync.dma_start(
                            out_sv[:, s0 : s0 + 4, :],
                            ctxT.rearrange("p c n s -> p (c n s)")
                            .rearrange("p (n h) -> p n h", n=4)
                            .bitcast(f32),
                        )
                        continue

                    # out-proj + bo + residual + LN1 -> y_all
                    wo_sb = pw.tile([128, 6, H], f32r, tag="wqkvo")
                    nc.sync.dma_start(
                        wo_sb,
                        wo_d.rearrange("(ko p) m -> p ko m", p=128).bitcast(f32r),
                    )
                    for si in range(4):
                        po = psAv.tile([128, H], f32, tag="pv")
                        for kc in range(6):
                            nc.tensor.matmul(
                                po[:, 0:512],
                                ctxT[:, kc, si, :],
                                wo_sb[:, kc, 0:512],
                                start=(kc == 0), stop=(kc == 5),
                            )
                        for kc in range(6):
                            nc.tensor.matmul(
                                po[:, 512:H],
                                ctxT[:, kc, si, :],
                                wo_sb[:, kc, 512:H],
                                start=(kc == 0), stop=(kc == 5),
                            )
                        z = pa2.tile([128, H], f32, tag="z")
                        nc.vector.tensor_add(z[:, 0:512], po[:, 0:512], bo_r[:, 0:512])
                        nc.vector.tensor_add(z[:, 512:H], po[:, 512:H], bo_r[:, 512:H])
                        nc.vector.tensor_add(z, z, x_g[:, si, :])
                        # LN1
                        st = pa2.tile([128, 3, 6], f32, tag="st")
                        zv = z.rearrange("p (a b) -> p a b", a=3)
                        for i in range(3):
                            nc.vector.bn_stats(st[:, i, :], zv[:, i, :])
                        mv = pa2.tile([128, 2], f32, tag="mv")
                        nc.vector.bn_aggr(mv, st)
                        sd = pa2.tile([128, 1], f32, tag="sd")
                        nc.scalar.activation(sd, mv[:, 1:2], AF.Sqrt, bias=eps_t[:, 0:1], scale=1.0)
                        nc.vector.reciprocal(sd, sd)
                        yslot = y_all[:, s0 + si, :]
                        nc.vector.tensor_scalar(
                            yslot, z,
                            scalar1=mv[:, 0:1], scalar2=sd,
                            op0=ALU.subtract, op1=ALU.mult,
                        )
                        nc.vector.tensor_mul(yslot, yslot, g1_r)
                        nc.vector.tensor_add(yslot, yslot, b1l_r)
                        for c in range(6):
                            pt = psAs.tile([128, 128], f32, tag="pt")
                            nc.tensor.transpose(
                                pt, yslot[:, c * 128 : (c + 1) * 128], ident
                            )
                            nc.vector.tensor_copy(yT_all[:, c, s0 + si, :], pt)

            if _STAGE == 1 and _SUB >= 4:
                nc.sync.dma_start(out_sv, y_all)
            # ---------------- Phase B: FFN + LN2 -> out ------------------
            with (
                tc.tile_pool(name="pb", bufs=1) as pb,
                tc.tile_pool(name="pb2", bufs=2) as pb2,
                tc.tile_pool(name="w2p", bufs=3) as w2p,
                tc.tile_pool(name="psB_a", bufs=1, space="PSUM") as psBa,
                tc.tile_pool(name="psB_g", bufs=2, space="PSUM") as psBg,
                tc.tile_pool(name="psB_t", bufs=1, space="PSUM") as psBt,
            ):
                for g in range(G if _STAGE >= 2 else 0):
                    s0 = g * 4
                    yT = yT_all[:, :, s0 : s0 + 4, :]

                    # w1 + gelu for the whole group: gT [128, 24, 4*128]
                    gT = pb.tile([128, 24, 512], f32r, tag="gT")
                    gelu_fn = (
                        AF.Identity if _SIM_GELU_IDENTITY else AF.Gelu_apprx_tanh
                    )
                    for sx in range(4):
                        w1q = pb2.tile([128, 6, 768], f32r, tag="w1q")
                        nc.sync.dma_start(
                            w1q,
                            w1_view[:, :, sx * 768 : (sx + 1) * 768].bitcast(f32r),
                        )
                        for fm in range(6):
                            pg = psBg.tile([128, 512], f32, tag="pg")
                            for kc in range(6):
                                nc.tensor.matmul(
                                    pg,
                                    w1q[:, kc, fm * 128 : (fm + 1) * 128],
                                    yT[:, kc, :, :],
                                    start=(kc == 0), stop=(kc == 5),
                                )
                            fg = sx * 6 + fm
                            nc.scalar.activation(
                                gT[:, fg, :], pg, gelu_fn,
                                bias=b1_sb[:, fg : fg + 1], scale=1.0,
                            )

                    # w2: two column passes; each streams its w2 columns once
                    z2_all = pb.tile([128, 4, H], f32, tag="z2_all")
                    for (c0, c1) in ((0, 512), (512, H)):
                        pw2 = [
                            psBa.tile([128, 512], f32, tag=f"pw2_{i}", name=f"pw2_{i}")
                            for i in range(4)
                        ]
                        for kc2 in range(12):
                            w2c = w2p.tile([128, 2, 512], f32r, tag="w2c")
                            nc.sync.dma_start(
                                w2c[:, :, : c1 - c0],
                                w2_d[kc2 * 256 : (kc2 + 1) * 256, c0:c1]
                                .rearrange("(a p) h -> p a h", p=128)
                                .bitcast(f32r),
                            )
                            for j in range(2):
                                kc = kc2 * 2 + j
                                for si in range(4):
                                    nc.tensor.matmul(
                                        pw2[si][:, : c1 - c0],
                                        gT[:, kc, si * 128 : (si + 1) * 128],
                                        w2c[:, j, : c1 - c0],
                                        start=(kc == 0), stop=(kc == 23),
                                    )
                        for si in range(4):
                            nc.vector.tensor_add(
                                z2_all[:, si, c0:c1],
                                pw2[si][:, : c1 - c0],
                                b2_r[:, c0:c1],
                            )

                    o_g = pb2.tile([128, 4, H], f32, tag="o_g")
                    for si in range(4):
                        z2 = z2_all[:, si, :]
                        nc.vector.tensor_add(z2, z2, y_all[:, s0 + si, :])
                        st = pb2.tile([128, 3, 6], f32, tag="stB")
                        z2v = z2.rearrange("p (a b) -> p a b", a=3)
                        for i in range(3):
                            nc.vector.bn_stats(st[:, i, :], z2v[:, i, :])
                        mv = pb2.tile([128, 2], f32, tag="mvB")
                        nc.vector.bn_aggr(mv, st)
                        sd = pb2.tile([128, 1], f32, tag="sdB")
                        nc.scalar.activation(sd, mv[:, 1:2], AF.Sqrt, bias=eps_t[:, 0:1], scale=1.0)
                        nc.vector.reciprocal(sd, sd)
                        oslot = o_g[:, si, :]
                        nc.vector.tensor_scalar(
                            oslot, z2,
                            scalar1=mv[:, 0:1], scalar2=sd,
                            op0=ALU.subtract, op1=ALU.mult,
                        )
                        nc.vector.tensor_mul(oslot, oslot, g2_r)
                        nc.vector.tensor_add(oslot, oslot, b2l_r)
                        nc.sync.dma_start(out_sv[:, s0 + si, :], oslot)


def _route_and_assign(hidden_states, centers):
    hp = hidden_states.mean(axis=1)  # [B, H]
    d2 = (
        (hp * hp).sum(-1, keepdims=True)
        - 2.0 * hp @ centers.T
        + (centers * centers).sum(-1)[None, :]
    )
    eid = np.argmin(d2, axis=1)  # [B]
    B = eid.shape[0]
    counts = np.bincount(eid, minlength=E)
    active = [e for e in range(E) if counts[e] > 0]
    # apportion cores to active experts proportionally (min 1 each)
    cores_e = {e: 1 for e in active}
    rem = NCORES - len(active)
    if rem > 0:
        quota = {e: counts[e] * NCORES / B for e in active}
        frac = {e: quota[e] - 1 for e in active}
        order = sorted(active, key=lambda e: -frac[e])
        whole = {e: max(0, int(np.floor(frac[e]))) for e in active}
        used = sum(whole.values())
        while used > rem:  # trim if overflow
            for e in sorted(active, key=lambda e: -whole[e]):
                if used <= rem:
                    break
                if whole[e] > 0:
                    whole[e] -= 1
                    used -= 1
        for e in active:
            cores_e[e] += whole[e]
        rem -= used
        i = 0
        frac_order = sorted(active, key=lambda e: -(frac[e] - whole[e]))
        while rem > 0:
            cores_e[frac_order[i % len(frac_order)]] += 1
            rem -= 1
            i += 1
    # assign sentences of each expert round-robin over its cores
    assign = [[] for _ in range(NCORES)]  # core -> list of batch idx
    core_expert = [active[0] if active else 0] * NCORES
    next_core = 0
    for e in active:
        ncr = cores_e[e]
        idxs = np.nonzero(eid == e)[0]
        chunks = np.array_split(idxs, ncr)
        for ch in chunks:
            assign[next_core] = list(ch)
            core_expert[next_core] = e
            next_core += 1
    max_load = max(len(a) for a in assign)
    nslot = max(4, int(np.ceil(max_load / 4.0)) * 4)
    return assign, core_expert, nslot


def kernel(**inputs):
    global LAST_RUN_WALL_NS
    import time

    from concourse.bass_utils import run_bass_kernel_spmd

    inputs = {k: np.ascontiguousarray(np.asarray(v)) for k, v in inputs.items()}
    hs = inputs["hidden_states"].astype(np.float32, copy=False)
    am = inputs["attention_mask"].astype(np.float32, copy=False)
    centers = inputs["centers"].astype(np.float32, copy=False)
    B = hs.shape[0]

    assign, core_expert, nslot = _route_and_assign(hs, centers)
    use_mask = bool(np.any(am != 0.0))

    key = (nslot, use_mask)
    if key not in _BUILD_CACHE:
        _BUILD_CACHE[key] = _build(nslot, use_mask)
    nc = _BUILD_CACHE[key]

    in_maps = []
    for c in range(NCORES):
        e = core_expert[c]
        idxs = assign[c]
        x = np.zeros((nslot, S, H), np.float32)
        m = np.zeros((nslot, S), np.float32)
        for j, b in enumerate(idxs):
            x[j] = hs[b]
            m[j] = am[b]
        im = {"x": x, "mask": m}
        for k in PARAM_KEYS:
            im[k] = np.ascontiguousarray(inputs[k][e])
        in_maps.append(im)

    t0 = time.perf_counter_ns()
    res = run_bass_kernel_spmd(nc, in_maps, core_ids=list(range(NCORES)))
    LAST_RUN_WALL_NS = time.perf_counter_ns() - t0

    out = np.zeros((B, S, H), np.float32)
    for c in range(NCORES):
        oc = res.results[c]["out"]
        for j, b in enumerate(assign[c]):
            out[b] = oc[j]
    return out



# revision 14
# speedup vs baseline: 14989.2320x; 14989.2320x over previous
"""MoE-routed transformer encoder layer on 8 Trainium2 cores.

Routing (mean -> nearest center -> expert id) is computed on host; sentences
are dispatched to cores so that each core runs exactly one expert's weights
over its share of sentences (expert/data parallelism, no device collectives).

Device kernel: dense encoder layer QKV -> attention -> out-proj -> LN1 ->
FFN(gelu) -> LN2. Weights and matmul operands are bf16 (full-rate PE, half
DMA/SBUF); PSUM accumulation, layernorm and softmax statistics stay fp32.
Weights are DMA'd once per phase and stay resident in SBUF across groups.
"""

import numpy as np

H = 768
NH = 12
HD = 64
FF = 3072
S = 128
E = 4
EPS = 1e-12
NCORES = 8

PARAM_KEYS = [
    "wq", "wk", "wv", "wo", "bq", "bk", "bv", "bo",
    "ln1_g", "ln1_b", "w1", "b1", "w2", "b2", "ln2_g", "ln2_b",
]
BF16_KEYS = {"wq", "wk", "wv", "wo", "w1", "w2"}

_BUILD_CACHE = {}
LAST_RUN_WALL_NS = None
LAST_RESULT = None  # BassKernelResults of the most recent run (for profiling)


def _build(nslot, use_mask):
    import concourse.bass as bass
    import concourse.mybir as mybir
    import concourse.tile as tile
    from concourse import bacc
    from concourse.masks import make_identity

    f32 = mybir.dt.float32
    bf16 = mybir.dt.bfloat16

    NS = nslot
    assert NS % 4 == 0
    G = NS // 4

    nc = bacc.Bacc("TRN2", target_bir_lowering=False, debug=False)

    x_d = nc.dram_tensor("x", [NS, S, H], f32, kind="ExternalInput").ap()
    mask_d = nc.dram_tensor("mask", [NS, S], f32, kind="ExternalInput").ap()
    wq_d = nc.dram_tensor("wq", [H, H], bf16, kind="ExternalInput").ap()
    wk_d = nc.dram_tensor("wk", [H, H], bf16, kind="ExternalInput").ap()
    wv_d = nc.dram_tensor("wv", [H, H], bf16, kind="ExternalInput").ap()
    wo_d = nc.dram_tensor("wo", [H, H], bf16, kind="ExternalInput").ap()
    bq_d = nc.dram_tensor("bq", [H], f32, kind="ExternalInput").ap()
    bk_d = nc.dram_tensor("bk", [H], f32, kind="ExternalInput").ap()
    bv_d = nc.dram_tensor("bv", [H], f32, kind="ExternalInput").ap()
    bo_d = nc.dram_tensor("bo", [H], f32, kind="ExternalInput").ap()
    g1_d = nc.dram_tensor("ln1_g", [H], f32, kind="ExternalInput").ap()
    b1l_d = nc.dram_tensor("ln1_b", [H], f32, kind="ExternalInput").ap()
    w1_d = nc.dram_tensor("w1", [H, FF], bf16, kind="ExternalInput").ap()
    b1_d = nc.dram_tensor("b1", [FF], f32, kind="ExternalInput").ap()
    w2_d = nc.dram_tensor("w2", [FF, H], bf16, kind="ExternalInput").ap()
    b2_d = nc.dram_tensor("b2", [H], f32, kind="ExternalInput").ap()
    g2_d = nc.dram_tensor("ln2_g", [H], f32, kind="ExternalInput").ap()
    b2l_d = nc.dram_tensor("ln2_b", [H], f32, kind="ExternalInput").ap()
    out_d = nc.dram_tensor("out", [NS, S, H], bf16, kind="ExternalOutput").ap()

    x_sv = x_d.rearrange("n s h -> s n h")       # partition dim = sequence pos
    out_sv = out_d.rearrange("n s h -> s n h")

    with tile.TileContext(nc) as tc:
        _kernel_body(
            nc, tc, bass, mybir, tile, make_identity, NS, G, use_mask,
            x_sv, out_sv, mask_d,
            wq_d, wk_d, wv_d, wo_d, bq_d, bk_d, bv_d, bo_d,
            g1_d, b1l_d, w1_d, b1_d, w2_d, b2_d, g2_d, b2l_d,
        )
    nc.compile()
    return nc


def _kernel_body(nc, tc, bass, mybir, tile, make_identity, NS, G, use_mask,
                 x_sv, out_sv, mask_d,
                 wq_d, wk_d, wv_d, wo_d, bq_d, bk_d, bv_d, bo_d,
                 g1_d, b1l_d, w1_d, b1_d, w2_d, b2_d, g2_d, b2l_d):
    f32 = mybir.dt.float32
    bf16 = mybir.dt.bfloat16
    AF = mybir.ActivationFunctionType
    ALU = mybir.AluOpType

    with (
        tc.tile_pool(name="const", bufs=1) as constp,
        tc.tile_pool(name="ybuf", bufs=1) as ybufp,
    ):
        ident = constp.tile([128, 128], f32)
        make_identity(nc, ident)
        eps_t = constp.tile([128, 1], f32)
        nc.vector.memset(eps_t, EPS)
        b1_sb = constp.tile([128, 24], f32)
        nc.gpsimd.dma_start(b1_sb, b1_d.rearrange("(o p) -> p o", p=128))

        def repl(pool, src, nm):
            t = pool.tile([128, H], f32, tag=nm, name=nm)
            bsrc = bass.AP(
                tensor=src.tensor, offset=src.offset, ap=[[0, 128], [1, H]]
            )
            nc.gpsimd.dma_start(t, bsrc)
            return t

        b2_r = repl(constp, b2_d, "b2_r")
        g2_r = repl(constp, g2_d, "g2_r")
        b2l_r = repl(constp, b2l_d, "b2l_r")
        y_all = ybufp.tile([128, NS, H], f32)
        yT_all = ybufp.tile([128, 6, NS, 128], bf16)

        # ---------------- Phase A: attention + LN1 -> y_all --------------
        with (
            tc.tile_pool(name="pa", bufs=1) as pa,
            tc.tile_pool(name="pa2", bufs=2) as pa2,
            tc.tile_pool(name="px", bufs=2) as px,
            tc.tile_pool(name="psA", bufs=4, space="PSUM") as psA,
            tc.tile_pool(name="psVO", bufs=2, space="PSUM") as psVO,
        ):
            bq_sb = pa.tile([128, 6], f32, tag="bq_sb", name="bq_sb")
            nc.gpsimd.dma_start(bq_sb, bq_d.rearrange("(o p) -> p o", p=128))
            bk_sb = pa.tile([128, 6], f32, tag="bk_sb", name="bk_sb")
            nc.gpsimd.dma_start(bk_sb, bk_d.rearrange("(o p) -> p o", p=128))
            bv_r = repl(pa, bv_d, "bv_r")
            bo_r = repl(pa, bo_d, "bo_r")
            g1_r = repl(pa, g1_d, "g1_r")
            b1l_r = repl(pa, b1l_d, "b1l_r")

            # per-phase resident weights (bf16, loaded once)
            wq_sb = pa.tile([128, 6, H], bf16, tag="wq_sb", name="wq_sb")
            nc.sync.dma_start(wq_sb, wq_d.rearrange("(ko p) m -> p ko m", p=128))
            wk_sb = pa.tile([128, 6, H], bf16, tag="wk_sb", name="wk_sb")
            nc.sync.dma_start(wk_sb, wk_d.rearrange("(ko p) m -> p ko m", p=128))
            wv_sb = pa.tile([128, 6, H], bf16, tag="wv_sb", name="wv_sb")
            nc.sync.dma_start(wv_sb, wv_d.rearrange("(ko p) m -> p ko m", p=128))
            wo_sb = pa.tile([128, 6, H], bf16, tag="wo_sb", name="wo_sb")
            nc.sync.dma_start(wo_sb, wo_d.rearrange("(ko p) m -> p ko m", p=128))

            for g in range(G):
                s0 = g * 4
                x_g = px.tile([128, 4, H], f32, tag="x_g")
                nc.sync.dma_start(x_g, x_sv[:, s0 : s0 + 4, :])
                if use_mask:
                    mrep = px.tile([128, 4, S], f32, tag="mrep")
                    src = bass.AP(
                        tensor=mask_d.tensor,
                        offset=s0 * S,
                        ap=[[0, 128], [S, 4], [1, S]],
                    )
                    nc.gpsimd.dma_start(mrep, src)

                # x transposed: xT[p, c, si, s] = x[s, si, c*128+p]
                xT = pa.tile([128, 6, 4, 128], bf16, tag="xT")
                for c in range(6):
                    pt4 = psA.tile([128, 512], f32, tag="pq", name="pt4")
                    for si in range(4):
                        nc.tensor.transpose(
                            pt4[:, si * 128 : (si + 1) * 128],
                            x_g[:, si, c * 128 : (c + 1) * 128],
                            ident,
                        )
                    nc.scalar.activation(xT[:, c, :, :], pt4, AF.Identity)

                # qT/kT: weight-stationary over 4-sentence pack (N=512)
                qT = pa.tile([128, 6, 4, 128], bf16, tag="qT")
                kT = pa.tile([128, 6, 4, 128], bf16, tag="kT")
                for w_sb, bias_sb, dstT in (
                    (wq_sb, bq_sb, qT),
                    (wk_sb, bk_sb, kT),
                ):
                    for mc in range(6):
                        pq = psA.tile([128, 512], f32, tag="pq", name="pq")
                        for kc in range(6):
                            nc.tensor.matmul(
                                pq,
                                w_sb[:, kc, mc * 128 : (mc + 1) * 128],
                                xT[:, kc, :, :],
                                start=(kc == 0),
                                stop=(kc == 5),
                            )
                        nc.scalar.activation(
                            dstT[:, mc, :, :],
                            pq,
                            AF.Identity,
                            bias=bias_sb[:, mc : mc + 1],
                            scale=1.0,
                        )

                # v in natural layout [s, 768]
                v_g = pa.tile([128, 4, H], bf16, tag="v_g")
                for si in range(4):
                    pv = psVO.tile([128, H], f32, tag="pv")
                    for c0, c1 in ((0, 512), (512, H)):
                        for kc in range(6):
                            nc.tensor.matmul(
                                pv[:, c0:c1],
                                xT[:, kc, si, :],
                                wv_sb[:, kc, c0:c1],
                                start=(kc == 0),
                                stop=(kc == 5),
                            )
                    nc.vector.tensor_add(v_g[:, si, :], pv, bv_r)

                # attention per sentence
                ctxT = pa.tile([128, 6, 4, 128], bf16, tag="xT")  # reuse xT slot
                for si in range(4):
                    attn = pa2.tile([128, NH, S], f32, tag="attn")
                    sums = pa2.tile([128, NH], f32, tag="sums")
                    for h in range(NH):
                        # one PSUM bank per head (PE-write while ACT-reads a
                        # shared bank is fatal on HW); head pairs pack into
                        # the PE array via tile_position and run concurrently
                        psc = psA.tile([128, 128], f32, tag="pq", name="psc")
                        nc.tensor.matmul(
                            psc,
                            qT[(h % 2) * 64 : (h % 2) * 64 + 64, h // 2, si, :],
                            kT[(h % 2) * 64 : (h % 2) * 64 + 64, h // 2, si, :],
                            start=True,
                            stop=True,
                            tile_position=((h % 2) * 64, 0),
                        )
                        if use_mask:
                            tmp = pa.tile([128, S], f32, tag="msk_tmp")
                            nc.vector.tensor_scalar_mul(tmp, psc, 0.125)
                            nc.vector.tensor_add(tmp, tmp, mrep[:, si, :])
                            nc.scalar.activation(
                                attn[:, h, :], tmp, AF.Exp,
                                bias=0.0, scale=1.0,
                                accum_out=sums[:, h : h + 1],
                            )
                        else:
                            nc.scalar.activation(
                                attn[:, h, :], psc, AF.Exp,
                                bias=0.0, scale=0.125,
                                accum_out=sums[:, h : h + 1],
                            )
                    rs = pa2.tile([128, NH], f32, tag="rs")
                    nc.vector.reciprocal(rs, sums)
                    for h in range(NH):
                        nc.vector.tensor_scalar_mul(
                            attn[:, h, :], attn[:, h, :], rs[:, h : h + 1]
                        )
                    attnT = pa2.tile([128, NH, S], bf16, tag="attnT")
                    for hg in range(3):
                        pt4 = psA.tile([128, 512], f32, tag="pq", name="pt4")
                        for j in range(4):
                            nc.tensor.transpose(
                                pt4[:, j * 128 : (j + 1) * 128],
                                attn[:, hg * 4 + j, :],
                                ident,
                            )
                        nc.scalar.activation(
                            attnT[:, hg * 4 : hg * 4 + 4, :], pt4, AF.Identity
                        )
                    for hq in range(2):  # 3 head-pairs per psum tile
                        pc3 = psA.tile([128, 512], f32, tag="pq", name="pc3")
                        for jp in range(3):
                            hp = hq * 3 + jp
                            nc.tensor.matmul(
                                pc3[0:64, jp * 128 : (jp + 1) * 128],
                                v_g[:, si, (2 * hp) * 64 : (2 * hp + 1) * 64],
                                attnT[:, 2 * hp, :],
                                start=True, stop=True,
                                tile_position=(0, 0),
                            )
                            nc.tensor.matmul(
                                pc3[64:128, jp * 128 : (jp + 1) * 128],
                                v_g[:, si, (2 * hp + 1) * 64 : (2 * hp + 2) * 64],
                                attnT[:, 2 * hp + 1, :],
                                start=True, stop=True,
                                tile_position=(0, 64),
                            )
                        nc.vector.tensor_copy(
                            ctxT[:, hq * 3 : hq * 3 + 3, si, :],
                            pc3[:, 0:384],
                        )

                # out-proj + bo + residual + LN1 -> y_all
                for si in range(4):
                    po = psVO.tile([128, H], f32, tag="pv")
                    for c0, c1 in ((0, 512), (512, H)):
                        for kc in range(6):
                            nc.tensor.matmul(
                                po[:, c0:c1],
                                ctxT[:, kc, si, :],
                                wo_sb[:, kc, c0:c1],
                                start=(kc == 0), stop=(kc == 5),
                            )
                    z = pa2.tile([128, H], f32, tag="z")
                    nc.vector.tensor_add(z, po, bo_r)
                    nc.vector.tensor_add(z, z, x_g[:, si, :])
                    # LN1 (stats in f32)
                    st = pa2.tile([128, 3, 6], f32, tag="st")
                    zv = z.rearrange("p (a b) -> p a b", a=3)
                    for i in range(3):
                        nc.vector.bn_stats(st[:, i, :], zv[:, i, :])
                    mv = pa2.tile([128, 2], f32, tag="mv")
                    nc.vector.bn_aggr(mv, st)
                    sd = pa2.tile([128, 1], f32, tag="sd")
                    nc.scalar.activation(
                        sd, mv[:, 1:2], AF.Sqrt, bias=eps_t[:, 0:1], scale=1.0
                    )
                    nc.vector.reciprocal(sd, sd)
                    nm = pa2.tile([128, 1], f32, tag="nm")
                    nc.vector.tensor_mul(nm, mv[:, 0:1], sd)
                    nc.vector.tensor_scalar_mul(nm, nm, -1.0)
                    y_f = pa2.tile([128, H], f32, tag="y_f")
                    nc.scalar.activation(
                        y_f, z, AF.Identity, bias=nm[:, 0:1], scale=sd[:, 0:1]
                    )
                    nc.gpsimd.tensor_mul(y_f, y_f, g1_r)
                    yslot = y_all[:, s0 + si, :]
                    nc.gpsimd.tensor_add(yslot, y_f, b1l_r)
                    for ch in range(2):
                        pt3 = psA.tile([128, 512], f32, tag="pq", name="pt3")
                        for j in range(3):
                            c = ch * 3 + j
                            nc.tensor.transpose(
                                pt3[:, j * 128 : (j + 1) * 128],
                                yslot[:, c * 128 : (c + 1) * 128],
                                ident,
                            )
                        nc.scalar.activation(
                            yT_all[:, ch * 3 : ch * 3 + 3, s0 + si, :],
                            pt3[:, 0:384],
                            AF.Identity,
                        )

        # ---------------- Phase B: FFN + LN2 -> out ----------------------
        with (
            tc.tile_pool(name="pb", bufs=1) as pb,
            tc.tile_pool(name="pb2", bufs=2) as pb2,
            tc.tile_pool(name="psBg", bufs=2, space="PSUM") as psBg,
            tc.tile_pool(name="psBw", bufs=2, space="PSUM") as psBw,
        ):
            w1_sb = pb.tile([128, 6, FF], bf16, tag="w1_sb", name="w1_sb")
            nc.sync.dma_start(w1_sb, w1_d.rearrange("(ko p) f -> p ko f", p=128))
            w2_sb = pb.tile([128, 24, H], bf16, tag="w2_sb", name="w2_sb")
            nc.sync.dma_start(w2_sb, w2_d.rearrange("(ko p) h -> p ko h", p=128))

            for g in range(G):
                s0 = g * 4
                # w1 + gelu for the whole group: gT [128, 24, 4*128]
                gT = pb.tile([128, 24, 512], bf16, tag="gT")
                for fg in range(24):
                    pg = psBg.tile([128, 512], f32, tag="pg")
                    for kc in range(6):
                        nc.tensor.matmul(
                            pg,
                            w1_sb[:, kc, fg * 128 : (fg + 1) * 128],
                            yT_all[:, kc, s0 : s0 + 4, :],
                            start=(kc == 0), stop=(kc == 5),
                        )
                    nc.scalar.activation(
                        gT[:, fg, :], pg, AF.Gelu_apprx_tanh,
                        bias=b1_sb[:, fg : fg + 1], scale=1.0,
                    )

                for si in range(4):
                    pw2 = psBw.tile([128, H], f32, tag="pw2")
                    for c0, c1 in ((0, 512), (512, H)):
                        for kc in range(24):
                            nc.tensor.matmul(
                                pw2[:, c0:c1],
                                gT[:, kc, si * 128 : (si + 1) * 128],
                                w2_sb[:, kc, c0:c1],
                                start=(kc == 0), stop=(kc == 23),
                            )
                    z2 = pb2.tile([128, H], f32, tag="z2")
                    nc.vector.tensor_add(z2, pw2, b2_r)
                    nc.vector.tensor_add(z2, z2, y_all[:, s0 + si, :])
                    st = pb2.tile([128, 3, 6], f32, tag="stB")
                    z2v = z2.rearrange("p (a b) -> p a b", a=3)
                    for i in range(3):
                        nc.vector.bn_stats(st[:, i, :], z2v[:, i, :])
                    mv = pb2.tile([128, 2], f32, tag="mvB")
                    nc.vector.bn_aggr(mv, st)
                    sd = pb2.tile([128, 1], f32, tag="sdB")
                    nc.scalar.activation(
                        sd, mv[:, 1:2], AF.Sqrt, bias=eps_t[:, 0:1], scale=1.0
                    )
                    nc.vector.reciprocal(sd, sd)
                    nm = pb2.tile([128, 1], f32, tag="nmB")
                    nc.vector.tensor_mul(nm, mv[:, 0:1], sd)
                    nc.vector.tensor_scalar_mul(nm, nm, -1.0)
                    o_f = pb2.tile([128, H], f32, tag="o_f")
                    nc.scalar.activation(
                        o_f, z2, AF.Identity, bias=nm[:, 0:1], scale=sd[:, 0:1]
                    )
                    nc.gpsimd.tensor_mul(o_f, o_f, g2_r)
                    o = pb2.tile([128, H], bf16, tag="o")
                    nc.vector.tensor_add(o, o_f, b2l_r)
                    nc.sync.dma_start(out_sv[:, s0 + si, :], o)


def _route_and_assign(hidden_states, centers):
    hp = hidden_states.mean(axis=1)  # [B, H]
    d2 = (
        (hp * hp).sum(-1, keepdims=True)
        - 2.0 * hp @ centers.T
        + (centers * centers).sum(-1)[None, :]
    )
    eid = np.argmin(d2, axis=1)  # [B]
    B = eid.shape[0]
    counts = np.bincount(eid, minlength=E)
    active = [e for e in range(E) if counts[e] > 0]
    # apportion cores to active experts proportionally (min 1 each)
    cores_e = {e: 1 for e in active}
    rem = NCORES - len(active)
    if rem > 0:
        quota = {e: counts[e] * NCORES / B for e in active}
        frac = {e: quota[e] - 1 for e in active}
        order = sorted(active, key=lambda e: -frac[e])
        whole = {e: max(0, int(np.floor(frac[e]))) for e in active}
        used = sum(whole.values())
        while used > rem:  # trim if overflow
            for e in sorted(active, key=lambda e: -whole[e]):
                if used <= rem:
                    break
                if whole[e] > 0:
                    whole[e] -= 1
                    used -= 1
        for e in active:
            cores_e[e] += whole[e]
        rem -= used
        i = 0
        frac_order = sorted(active, key=lambda e: -(frac[e] - whole[e]))
        while rem > 0:
            cores_e[frac_order[i % len(frac_order)]] += 1
            rem -= 1
            i += 1
    # assign sentences of each expert round-robin over its cores
    assign = [[] for _ in range(NCORES)]  # core -> list of batch idx
    core_expert = [active[0] if active else 0] * NCORES
    next_core = 0
    for e in active:
        ncr = cores_e[e]
        idxs = np.nonzero(eid == e)[0]
        chunks = np.array_split(idxs, ncr)
        for ch in chunks:
            assign[next_core] = list(ch)
            core_expert[next_core] = e
            next_core += 1
    max_load = max(len(a) for a in assign)
    nslot = max(4, int(np.ceil(max_load / 4.0)) * 4)
    return assign, core_expert, nslot


def kernel(**inputs):
    global LAST_RUN_WALL_NS, LAST_RESULT
    import time

    import ml_dtypes
    from concourse.bass_utils import run_bass_kernel_spmd

    bf16 = ml_dtypes.bfloat16
    inputs = {k: np.ascontiguousarray(np.asarray(v)) for k, v in inputs.items()}
    hs = inputs["hidden_states"].astype(np.float32, copy=False)
    am = inputs["attention_mask"].astype(np.float32, copy=False)
    centers = inputs["centers"].astype(np.float32, copy=False)
    B = hs.shape[0]

    assign, core_expert, nslot = _route_and_assign(hs, centers)
    use_mask = bool(np.any(am != 0.0))

    key = (nslot, use_mask)
    if key not in _BUILD_CACHE:
        _BUILD_CACHE[key] = _build(nslot, use_mask)
    nc = _BUILD_CACHE[key]

    # convert each expert's big weights to bf16 once (reused by its cores)
    wcast = {
        k: [np.ascontiguousarray(inputs[k][e].astype(bf16)) for e in range(E)]
        for k in PARAM_KEYS if k in BF16_KEYS
    }
    in_maps = []
    for c in range(NCORES):
        e = core_expert[c]
        idxs = assign[c]
        x = np.zeros((nslot, S, H), np.float32)
        m = np.zeros((nslot, S), np.float32)
        for j, b in enumerate(idxs):
            x[j] = hs[b]
            m[j] = am[b]
        im = {"x": x, "mask": m}
        for k in PARAM_KEYS:
            if k in BF16_KEYS:
                im[k] = wcast[k][e]
            else:
                im[k] = np.ascontiguousarray(inputs[k][e])
        in_maps.append(im)

    t0 = time.perf_counter_ns()
    res = run_bass_kernel_spmd(nc, in_maps, core_ids=list(range(NCORES)))
    LAST_RUN_WALL_NS = time.perf_counter_ns() - t0
    LAST_RESULT = res

    out = np.zeros((B, S, H), np.float32)
    for c in range(NCORES):
        oc = res.results[c]["out"]
        for j, b in enumerate(assign[c]):
            out[b] = oc[j].astype(np.float32)
    return out


# revision 19
# speedup vs baseline: 15001.6327x; 1.0008x over previous
"""MoE-routed transformer encoder layer on 8 Trainium2 cores.

Routing (mean -> nearest center -> expert id) is computed on host; sentences
are dispatched to cores so that each core runs exactly one expert's weights
over its share of sentences (expert/data parallelism, no device collectives).

Device kernel: dense encoder layer QKV -> attention -> out-proj -> LN1 ->
FFN(gelu) -> LN2. Weights and matmul operands are bf16 (full-rate PE, half
DMA/SBUF); PSUM accumulation, layernorm and softmax statistics stay fp32.
Weights are DMA'd once per phase and stay resident in SBUF; each stationary
is loaded into the PE array once (standalone ldweights + non-self-loading
matmuls inside tile_critical) and amortized over all sentences.
"""

import numpy as np

H = 768
NH = 12
HD = 64
FF = 3072
S = 128
E = 4
EPS = 1e-12
NCORES = 8

PARAM_KEYS = [
    "wq", "wk", "wv", "wo", "bq", "bk", "bv", "bo",
    "ln1_g", "ln1_b", "w1", "b1", "w2", "b2", "ln2_g", "ln2_b",
]
BF16_KEYS = {"wq", "wk", "wv", "wo", "w1", "w2"}

_BUILD_CACHE = {}
LAST_RUN_WALL_NS = None
LAST_RESULT = None  # BassKernelResults of the most recent run (for profiling)


def _build(nslot, use_mask):
    import concourse.bass as bass
    import concourse.mybir as mybir
    import concourse.tile as tile
    from concourse import bacc
    from concourse.masks import make_identity

    f32 = mybir.dt.float32
    bf16 = mybir.dt.bfloat16

    NS = nslot
    assert NS % 4 == 0
    G = NS // 4

    nc = bacc.Bacc("TRN2", target_bir_lowering=False, debug=False)

    x_d = nc.dram_tensor("x", [NS, S, H], f32, kind="ExternalInput").ap()
    mask_d = nc.dram_tensor("mask", [NS, S], f32, kind="ExternalInput").ap()
    wq_d = nc.dram_tensor("wq", [H, H], bf16, kind="ExternalInput").ap()
    wk_d = nc.dram_tensor("wk", [H, H], bf16, kind="ExternalInput").ap()
    wv_d = nc.dram_tensor("wv", [H, H], bf16, kind="ExternalInput").ap()
    wo_d = nc.dram_tensor("wo", [H, H], bf16, kind="ExternalInput").ap()
    bq_d = nc.dram_tensor("bq", [H], f32, kind="ExternalInput").ap()
    bk_d = nc.dram_tensor("bk", [H], f32, kind="ExternalInput").ap()
    bv_d = nc.dram_tensor("bv", [H], f32, kind="ExternalInput").ap()
    bo_d = nc.dram_tensor("bo", [H], f32, kind="ExternalInput").ap()
    g1_d = nc.dram_tensor("ln1_g", [H], f32, kind="ExternalInput").ap()
    b1l_d = nc.dram_tensor("ln1_b", [H], f32, kind="ExternalInput").ap()
    w1_d = nc.dram_tensor("w1", [H, FF], bf16, kind="ExternalInput").ap()
    b1_d = nc.dram_tensor("b1", [FF], f32, kind="ExternalInput").ap()
    w2_d = nc.dram_tensor("w2", [FF, H], bf16, kind="ExternalInput").ap()
    b2_d = nc.dram_tensor("b2", [H], f32, kind="ExternalInput").ap()
    g2_d = nc.dram_tensor("ln2_g", [H], f32, kind="ExternalInput").ap()
    b2l_d = nc.dram_tensor("ln2_b", [H], f32, kind="ExternalInput").ap()
    out_d = nc.dram_tensor("out", [NS, S, H], bf16, kind="ExternalOutput").ap()

    x_sv = x_d.rearrange("n s h -> s n h")       # partition dim = sequence pos
    out_sv = out_d.rearrange("n s h -> s n h")

    with tile.TileContext(nc) as tc:
        _kernel_body(
            nc, tc, bass, mybir, tile, make_identity, NS, G, use_mask,
            x_sv, out_sv, mask_d,
            wq_d, wk_d, wv_d, wo_d, bq_d, bk_d, bv_d, bo_d,
            g1_d, b1l_d, w1_d, b1_d, w2_d, b2_d, g2_d, b2l_d,
        )
    nc.compile()
    return nc


def _kernel_body(nc, tc, bass, mybir, tile, make_identity, NS, G, use_mask,
                 x_sv, out_sv, mask_d,
                 wq_d, wk_d, wv_d, wo_d, bq_d, bk_d, bv_d, bo_d,
                 g1_d, b1l_d, w1_d, b1_d, w2_d, b2_d, g2_d, b2l_d):
    f32 = mybir.dt.float32
    bf16 = mybir.dt.bfloat16
    AF = mybir.ActivationFunctionType
    ALU = mybir.AluOpType

    def mm_unit(stat, mms):
        """Same-stationary matmuls emitted back-to-back; walrus's ldw-opt
        pass (if enabled) elides the redundant PE weight reloads."""
        for out, mov, start, stop in mms:
            nc.tensor.matmul(out, stat, mov, start=start, stop=stop)

    with (
        tc.tile_pool(name="const", bufs=1) as constp,
        tc.tile_pool(name="ybuf", bufs=1) as ybufp,
    ):
        ident = constp.tile([128, 128], f32)
        make_identity(nc, ident)
        eps_t = constp.tile([128, 1], f32)
        nc.vector.memset(eps_t, EPS)
        b1_sb = constp.tile([128, 24], f32)
        nc.gpsimd.dma_start(b1_sb, b1_d.rearrange("(o p) -> p o", p=128))

        def repl(pool, src, nm):
            t = pool.tile([128, H], f32, tag=nm, name=nm)
            bsrc = bass.AP(
                tensor=src.tensor, offset=src.offset, ap=[[0, 128], [1, H]]
            )
            nc.gpsimd.dma_start(t, bsrc)
            return t

        b2_r = repl(constp, b2_d, "b2_r")
        g2_r = repl(constp, g2_d, "g2_r")
        b2l_r = repl(constp, b2l_d, "b2l_r")
        y_all = ybufp.tile([128, NS, H], f32)
        yT_all = ybufp.tile([128, 6, NS, 128], bf16)

        # ---------------- Phase A: attention + LN1 -> y_all --------------
        with (
            tc.tile_pool(name="pa", bufs=1) as pa,
            tc.tile_pool(name="pa2", bufs=2) as pa2,
            tc.tile_pool(name="px", bufs=1) as px,
            tc.tile_pool(name="pxr", bufs=2) as pxr,
            tc.tile_pool(name="psA", bufs=4, space="PSUM") as psA,
            tc.tile_pool(name="psVO", bufs=2, space="PSUM") as psVO,
        ):
            bq_sb = pa.tile([128, 6], f32, tag="bq_sb", name="bq_sb")
            nc.gpsimd.dma_start(bq_sb, bq_d.rearrange("(o p) -> p o", p=128))
            bk_sb = pa.tile([128, 6], f32, tag="bk_sb", name="bk_sb")
            nc.gpsimd.dma_start(bk_sb, bk_d.rearrange("(o p) -> p o", p=128))
            bv_r = repl(pa, bv_d, "bv_r")
            bo_r = repl(pa, bo_d, "bo_r")
            g1_r = repl(pa, g1_d, "g1_r")
            b1l_r = repl(pa, b1l_d, "b1l_r")

            # per-phase resident weights (bf16, loaded once, sync queue)
            wq_sb = pa.tile([128, 6, H], bf16, tag="wq_sb", name="wq_sb")
            nc.sync.dma_start(wq_sb, wq_d.rearrange("(ko p) m -> p ko m", p=128))
            wk_sb = pa.tile([128, 6, H], bf16, tag="wk_sb", name="wk_sb")
            nc.sync.dma_start(wk_sb, wk_d.rearrange("(ko p) m -> p ko m", p=128))
            wv_sb = pa.tile([128, 6, H], bf16, tag="wv_sb", name="wv_sb")
            nc.sync.dma_start(wv_sb, wv_d.rearrange("(ko p) m -> p ko m", p=128))
            wo_sb = pa.tile([128, 6, H], bf16, tag="wo_sb", name="wo_sb")
            nc.sync.dma_start(wo_sb, wo_d.rearrange("(ko p) m -> p ko m", p=128))

            # x transposed for all sentences: xT[p, c, s, :] (bf16)
            xT_all = pa.tile([128, 6, NS, 128], bf16, tag="xT")
            for g in range(G):
                s0 = g * 4
                x_g = px.tile([128, 4, H], f32, tag="x_g")
                nc.scalar.dma_start(x_g, x_sv[:, s0 : s0 + 4, :])
                for c in range(6):
                    pt4 = psA.tile([128, 512], f32, tag="pq", name="pt4")
                    for si in range(4):
                        nc.tensor.transpose(
                            pt4[:, si * 128 : (si + 1) * 128],
                            x_g[:, si, c * 128 : (c + 1) * 128],
                            ident,
                        )
                    nc.vector.tensor_copy(xT_all[:, c, s0 : s0 + 4, :], pt4)

            # qT/kT for all sentences: stationary loaded once per (w, mc, kc)
            qT = pa.tile([128, 6, NS, 128], bf16, tag="qT")
            kT = pa.tile([128, 6, NS, 128], bf16, tag="kT")
            for w_sb, bias_sb, dstT in (
                (wq_sb, bq_sb, qT),
                (wk_sb, bk_sb, kT),
            ):
                for mc in range(6):
                    pq = [
                        psA.tile([128, 512], f32, tag="pq", name=f"pq{g}")
                        for g in range(G)
                    ]
                    for kc in range(6):
                        mm_unit(
                            w_sb[:, kc, mc * 128 : (mc + 1) * 128],
                            [
                                (pq[g], xT_all[:, kc, 4 * g : 4 * g + 4, :],
                                 kc == 0, kc == 5)
                                for g in range(G)
                            ],
                        )
                    for g in range(G):
                        nc.scalar.activation(
                            dstT[:, mc, 4 * g : 4 * g + 4, :],
                            pq[g],
                            AF.Identity,
                            bias=bias_sb[:, mc : mc + 1],
                            scale=1.0,
                        )

            # v in natural layout [s, 768] for all sentences
            v_all = pa.tile([128, NS, H], bf16, tag="v_all")
            for s in range(NS):
                pv = psVO.tile([128, H], f32, tag="pv")
                for kc in range(6):
                    mm_unit(
                        xT_all[:, kc, s, :],
                        [
                            (pv[:, 0:512], wv_sb[:, kc, 0:512], kc == 0, kc == 5),
                            (pv[:, 512:H], wv_sb[:, kc, 512:H], kc == 0, kc == 5),
                        ],
                    )
                nc.vector.tensor_add(v_all[:, s, :], pv, bv_r)

            # attention per sentence -> ctxT (reuses xT slot)
            ctxT = pa.tile([128, 6, NS, 128], bf16, tag="xT")
            for s in range(NS):
                if use_mask:
                    mrep = pxr.tile([128, S], f32, tag="mrep")
                    src = bass.AP(
                        tensor=mask_d.tensor,
                        offset=s * S,
                        ap=[[0, 128], [1, S]],
                    )
                    nc.gpsimd.dma_start(mrep, src)
                attn = pa2.tile([128, NH, S], f32, tag="attn")
                sums = pa2.tile([128, NH], f32, tag="sums")
                for h in range(NH):
                    # one PSUM bank per head (PE-write while ACT-reads a
                    # shared bank is fatal on HW); head pairs pack into the
                    # PE array via tile_position and run concurrently
                    psc = psA.tile([128, 128], f32, tag="pq", name="psc")
                    nc.tensor.matmul(
                        psc,
                        qT[(h % 2) * 64 : (h % 2) * 64 + 64, h // 2, s, :],
                        kT[(h % 2) * 64 : (h % 2) * 64 + 64, h // 2, s, :],
                        start=True,
                        stop=True,
                        tile_position=((h % 2) * 64, 0),
                    )
                    if use_mask:
                        tmp = pa2.tile([128, S], f32, tag="msk_tmp")
                        nc.vector.tensor_scalar_mul(tmp, psc, 0.125)
                        nc.vector.tensor_add(tmp, tmp, mrep)
                        nc.scalar.activation(
                            attn[:, h, :], tmp, AF.Exp,
                            bias=0.0, scale=1.0,
                            accum_out=sums[:, h : h + 1],
                        )
                    else:
                        nc.scalar.activation(
                            attn[:, h, :], psc, AF.Exp,
                            bias=0.0, scale=0.125,
                            accum_out=sums[:, h : h + 1],
                        )
                rs = pa2.tile([128, NH], f32, tag="rs")
                nc.vector.reciprocal(rs, sums)
                for h in range(NH):
                    nc.vector.tensor_scalar_mul(
                        attn[:, h, :], attn[:, h, :], rs[:, h : h + 1]
                    )
                attnT = pa2.tile([128, NH, S], bf16, tag="attnT")
                for hg in range(3):
                    pt4 = psA.tile([128, 512], f32, tag="pq", name="pt4")
                    for j in range(4):
                        nc.tensor.transpose(
                            pt4[:, j * 128 : (j + 1) * 128],
                            attn[:, hg * 4 + j, :],
                            ident,
                        )
                    nc.scalar.activation(
                        attnT[:, hg * 4 : hg * 4 + 4, :], pt4, AF.Identity
                    )
                for hq in range(2):  # 3 head-pairs per psum tile
                    pc3 = psA.tile([128, 512], f32, tag="pq", name="pc3")
                    for jp in range(3):
                        hp = hq * 3 + jp
                        nc.tensor.matmul(
                            pc3[0:64, jp * 128 : (jp + 1) * 128],
                            v_all[:, s, (2 * hp) * 64 : (2 * hp + 1) * 64],
                            attnT[:, 2 * hp, :],
                            start=True, stop=True,
                            tile_position=(0, 0),
                        )
                        nc.tensor.matmul(
                            pc3[64:128, jp * 128 : (jp + 1) * 128],
                            v_all[:, s, (2 * hp + 1) * 64 : (2 * hp + 2) * 64],
                            attnT[:, 2 * hp + 1, :],
                            start=True, stop=True,
                            tile_position=(0, 64),
                        )
                    nc.vector.tensor_copy(
                        ctxT[:, hq * 3 : hq * 3 + 3, s, :],
                        pc3[:, 0:384],
                    )

            # out-proj + bo + residual + LN1 -> y_all
            for s in range(NS):
                x_res = pxr.tile([128, H], f32, tag="x_res")
                nc.scalar.dma_start(x_res, x_sv[:, s, :])
                po = psVO.tile([128, H], f32, tag="pv")
                for kc in range(6):
                    mm_unit(
                        ctxT[:, kc, s, :],
                        [
                            (po[:, 0:512], wo_sb[:, kc, 0:512], kc == 0, kc == 5),
                            (po[:, 512:H], wo_sb[:, kc, 512:H], kc == 0, kc == 5),
                        ],
                    )
                z = pa2.tile([128, H], f32, tag="z")
                nc.vector.tensor_add(z, po, bo_r)
                nc.vector.tensor_add(z, z, x_res)
                # LN1 (stats in f32)
                st = pa2.tile([128, 3, 6], f32, tag="st")
                zv = z.rearrange("p (a b) -> p a b", a=3)
                for i in range(3):
                    nc.vector.bn_stats(st[:, i, :], zv[:, i, :])
                mv = pa2.tile([128, 2], f32, tag="mv")
                nc.vector.bn_aggr(mv, st)
                sd = pa2.tile([128, 1], f32, tag="sd")
                nc.scalar.activation(
                    sd, mv[:, 1:2], AF.Sqrt, bias=eps_t[:, 0:1], scale=1.0
                )
                nc.vector.reciprocal(sd, sd)
                nm = pa2.tile([128, 1], f32, tag="nm")
                nc.vector.tensor_mul(nm, mv[:, 0:1], sd)
                nc.vector.tensor_scalar_mul(nm, nm, -1.0)
                y_f = pa2.tile([128, H], f32, tag="y_f")
                nc.scalar.activation(
                    y_f, z, AF.Identity, bias=nm[:, 0:1], scale=sd[:, 0:1]
                )
                nc.gpsimd.tensor_mul(y_f, y_f, g1_r)
                yslot = y_all[:, s, :]
                nc.gpsimd.tensor_add(yslot, y_f, b1l_r)
                for ch in range(2):
                    pt3 = psA.tile([128, 512], f32, tag="pq", name="pt3")
                    for j in range(3):
                        c = ch * 3 + j
                        nc.tensor.transpose(
                            pt3[:, j * 128 : (j + 1) * 128],
                            yslot[:, c * 128 : (c + 1) * 128],
                            ident,
                        )
                    nc.vector.tensor_copy(
                        yT_all[:, ch * 3 : ch * 3 + 3, s, :],
                        pt3[:, 0:384],
                    )

        # ---------------- Phase B: FFN + LN2 -> out ----------------------
        with (
            tc.tile_pool(name="pb", bufs=1) as pb,
            tc.tile_pool(name="pb2", bufs=2) as pb2,
            tc.tile_pool(name="psBg", bufs=4, space="PSUM") as psBg,
            tc.tile_pool(name="psBw", bufs=2, space="PSUM") as psBw,
        ):
            w1_sb = pb.tile([128, 6, FF], bf16, tag="w1_sb", name="w1_sb")
            nc.sync.dma_start(w1_sb, w1_d.rearrange("(ko p) f -> p ko f", p=128))
            w2_sb = pb.tile([128, 24, H], bf16, tag="w2_sb", name="w2_sb")
            nc.sync.dma_start(w2_sb, w2_d.rearrange("(ko p) h -> p ko h", p=128))

            # w1 + gelu for all groups: gT[g] [128, 24, 4*128]
            gT = [
                pb.tile([128, 24, 512], bf16, tag=f"gT{g}", name=f"gT{g}")
                for g in range(G)
            ]
            for fg in range(24):
                pg = [
                    psBg.tile([128, 512], f32, tag="pg", name=f"pg{g}")
                    for g in range(G)
                ]
                for kc in range(6):
                    mm_unit(
                        w1_sb[:, kc, fg * 128 : (fg + 1) * 128],
                        [
                            (pg[g], yT_all[:, kc, 4 * g : 4 * g + 4, :],
                             kc == 0, kc == 5)
                            for g in range(G)
                        ],
                    )
                for g in range(G):
                    nc.scalar.activation(
                        gT[g][:, fg, :], pg[g], AF.Gelu_apprx_tanh,
                        bias=b1_sb[:, fg : fg + 1], scale=1.0,
                    )

            for s in range(NS):
                g, si = s // 4, s % 4
                pw2 = psBw.tile([128, H], f32, tag="pw2")
                for kc in range(24):
                    mm_unit(
                        gT[g][:, kc, si * 128 : (si + 1) * 128],
                        [
                            (pw2[:, 0:512], w2_sb[:, kc, 0:512],
                             kc == 0, kc == 23),
                            (pw2[:, 512:H], w2_sb[:, kc, 512:H],
                             kc == 0, kc == 23),
                        ],
                    )
                z2 = pb2.tile([128, H], f32, tag="z2")
                nc.vector.tensor_add(z2, pw2, b2_r)
                nc.vector.tensor_add(z2, z2, y_all[:, s, :])
                st = pb2.tile([128, 3, 6], f32, tag="stB")
                z2v = z2.rearrange("p (a b) -> p a b", a=3)
                for i in range(3):
                    nc.vector.bn_stats(st[:, i, :], z2v[:, i, :])
                mv = pb2.tile([128, 2], f32, tag="mvB")
                nc.vector.bn_aggr(mv, st)
                sd = pb2.tile([128, 1], f32, tag="sdB")
                nc.scalar.activation(
                    sd, mv[:, 1:2], AF.Sqrt, bias=eps_t[:, 0:1], scale=1.0
                )
                nc.vector.reciprocal(sd, sd)
                nm = pb2.tile([128, 1], f32, tag="nmB")
                nc.vector.tensor_mul(nm, mv[:, 0:1], sd)
                nc.vector.tensor_scalar_mul(nm, nm, -1.0)
                o_f = pb2.tile([128, H], f32, tag="o_f")
                nc.scalar.activation(
                    o_f, z2, AF.Identity, bias=nm[:, 0:1], scale=sd[:, 0:1]
                )
                nc.gpsimd.tensor_mul(o_f, o_f, g2_r)
                o = pb2.tile([128, H], bf16, tag="o")
                nc.vector.tensor_add(o, o_f, b2l_r)
                nc.scalar.dma_start(out_sv[:, s, :], o)


def _route_and_assign(hidden_states, centers):
    hp = hidden_states.mean(axis=1)  # [B, H]
    d2 = (
        (hp * hp).sum(-1, keepdims=True)
        - 2.0 * hp @ centers.T
        + (centers * centers).sum(-1)[None, :]
    )
    eid = np.argmin(d2, axis=1)  # [B]
    B = eid.shape[0]
    counts = np.bincount(eid, minlength=E)
    active = [e for e in range(E) if counts[e] > 0]
    # apportion cores to active experts proportionally (min 1 each)
    cores_e = {e: 1 for e in active}
    rem = NCORES - len(active)
    if rem > 0:
        quota = {e: counts[e] * NCORES / B for e in active}
        frac = {e: quota[e] - 1 for e in active}
        order = sorted(active, key=lambda e: -frac[e])
        whole = {e: max(0, int(np.floor(frac[e]))) for e in active}
        used = sum(whole.values())
        while used > rem:  # trim if overflow
            for e in sorted(active, key=lambda e: -whole[e]):
                if used <= rem:
                    break
                if whole[e] > 0:
                    whole[e] -= 1
                    used -= 1
        for e in active:
            cores_e[e] += whole[e]
        rem -= used
        i = 0
        frac_order = sorted(active, key=lambda e: -(frac[e] - whole[e]))
        while rem > 0:
            cores_e[frac_order[i % len(frac_order)]] += 1
            rem -= 1
            i += 1
    # assign sentences of each expert round-robin over its cores
    assign = [[] for _ in range(NCORES)]  # core -> list of batch idx
    core_expert = [active[0] if active else 0] * NCORES
    next_core = 0
    for e in active:
        ncr = cores_e[e]
        idxs = np.nonzero(eid == e)[0]
        chunks = np.array_split(idxs, ncr)
        for ch in chunks:
            assign[next_core] = list(ch)
            core_expert[next_core] = e
            next_core += 1
    max_load = max(len(a) for a in assign)
    nslot = max(4, int(np.ceil(max_load / 4.0)) * 4)
    return assign, core_expert, nslot


_LDW_PATCHED = False


def _maybe_enable_ldw_opt():
    """Optionally let walrus elide redundant PE weight reloads (post-schedule
    pass, sound w.r.t. final instruction order). Gated for A/B testing."""
    global _LDW_PATCHED
    import os

    if _LDW_PATCHED or os.environ.get("BASS_TRY_LDW_OPT") != "1":
        return
    import concourse.bass_utils as bu

    orig = bu.run_command

    def run_patched(argv, **kw):
        argv = [
            "--enable-ldw-opt=true" if a == "--enable-ldw-opt=false" else a
            for a in argv
        ]
        return orig(argv, **kw)

    bu.run_command = run_patched
    _LDW_PATCHED = True


def kernel(**inputs):
    global LAST_RUN_WALL_NS, LAST_RESULT
    import time

    import ml_dtypes
    from concourse.bass_utils import run_bass_kernel_spmd

    _maybe_enable_ldw_opt()

    bf16 = ml_dtypes.bfloat16
    inputs = {k: np.ascontiguousarray(np.asarray(v)) for k, v in inputs.items()}
    hs = inputs["hidden_states"].astype(np.float32, copy=False)
    am = inputs["attention_mask"].astype(np.float32, copy=False)
    centers = inputs["centers"].astype(np.float32, copy=False)
    B = hs.shape[0]

    assign, core_expert, nslot = _route_and_assign(hs, centers)
    use_mask = bool(np.any(am != 0.0))

    key = (nslot, use_mask)
    if key not in _BUILD_CACHE:
        _BUILD_CACHE[key] = _build(nslot, use_mask)
    nc = _BUILD_CACHE[key]

    # convert each expert's big weights to bf16 once (reused by its cores)
    wcast = {
        k: [np.ascontiguousarray(inputs[k][e].astype(bf16)) for e in range(E)]
        for k in PARAM_KEYS if k in BF16_KEYS
    }
    in_maps = []
    for c in range(NCORES):
        e = core_expert[c]
        idxs = assign[c]
        x = np.zeros((nslot, S, H), np.float32)
        m = np.zeros((nslot, S), np.float32)
        for j, b in enumerate(idxs):
            x[j] = hs[b]
            m[j] = am[b]
        im = {"x": x, "mask": m}
        for k in PARAM_KEYS:
            if k in BF16_KEYS:
                im[k] = wcast[k][e]
            else:
                im[k] = np.ascontiguousarray(inputs[k][e])
        in_maps.append(im)

    t0 = time.perf_counter_ns()
    res = run_bass_kernel_spmd(nc, in_maps, core_ids=list(range(NCORES)))
    LAST_RUN_WALL_NS = time.perf_counter_ns() - t0
    LAST_RESULT = res

    out = np.zeros((B, S, H), np.float32)
    for c in range(NCORES):
        oc = res.results[c]["out"]
        for j, b in enumerate(assign[c]):
            out[b] = oc[j].astype(np.float32)
    return out
